# revision 1
# baseline (speedup 1.0000x reference)
"""Trainium2 Bass kernel for a grouped contrastive loss.

Math (matches the reference):
    z_a = concat(z_target, z_source)                      # [A=M+N, D]
    sims[a, j] = (z_a[a] . z_target[j]) / T
    den[j]  = sum_a exp(sims[a, j]) - exp(z_tj.z_tj / T)
    num[j]  = mean_{s: seg_source[s]==seg_target[j]} (z_s . z_tj) / T
            = (v_j . z_tj)   with v_j = S[seg_target[j]] / (count * T),
              S[g] = sum of z_source rows in group g       (exact linearity)
    loss = sum_j log(den[j]) - num[j]

Sharding: target columns j split across 8 cores (512 each); z_a replicated
(per the sharding hint). Per core, per 128-column block jb and 2048-wide
a-range g: PE matmuls (fp16 in, fp32 accum) fill a 4-bank PSUM tile, and one
ScalarE Exp activation with accum_out produces the per-column partial
exp-sums directly (cols of `res`). DVE computes the numerator partial
sum_j v'_j . z_tj (res2). ACT is the bottleneck engine (~1 elem/cycle/lane
for 33.5M exps across 8 cores); matmuls and DMAs hide underneath it.

The self term exp(z_tj.z_tj/T) ~ exp(1/T) ~ 1.6e6 dwarfs den ~ 1.8e4, so the
host must subtract (nearly) exactly what the device folded into the exp sums:
with fp16 inputs each PE product is exact in fp32, and np.sum's fp32 pairwise
accumulation reproduces the PE accumulator to ~2 ulp (verified on hardware),
which keeps the cancellation residual at ~1e-5 of the loss.

Host: tiny final reduction (log over 4096 columns + scalar sums) in float64.
"""

import numpy as np

TEMPERATURE = 0.07
N = 4096       # z_source rows
M = 4096       # z_target rows
D = 128        # embedding dim
G = 64         # groups
NCORES = 8
MLOC = M // NCORES          # 512 target columns per core
A = M + N                   # 8192 rows of z_a
ACH = 512                   # a-chunk (one matmul rhs / one PSUM bank)
NCH = A // ACH              # 16 chunks
GRP = 4                     # chunks per ACT group -> 2048-wide exp+accum
NGRP = NCH // GRP           # 4 groups per jb
NJB = MLOC // 128           # 4 column blocks of 128 per core
SPLITS = {}   # ACT subdivision (was used to absorb early DMA bubbles; now stale)


def _layout():
    """Each group's exp row-sum goes to one of two sinks: ACT accum_out
    (res, odd groups — includes the final group so the tail stays short) or
    a DVE tensor_reduce (res2 cols >= 1, even groups — saves the 187ns ACT
    accumulator-read per group). res2 is written only by DVE; res only by
    ACT: no cross-engine tile sharing."""
    acc_cols = {}
    dve_cols = {}
    ca, cd = 0, 1          # res2 col 0 = num partial
    dve_set = {0, 1, 2, 3, 4, 6, 7, 8, 9, 10, 12, 13}   # not 14/15: DVE must
    for jb in range(NJB):                          # finish before ACT does
        for g in range(NGRP):
            n = SPLITS.get((jb, g), 1)
            if jb * NGRP + g in dve_set:
                dve_cols[(jb, g)] = (cd, n)
                cd += n
            else:
                acc_cols[(jb, g)] = (ca, n)
                ca += n
    return acc_cols, dve_cols, ca, cd


_ACC_COLS, _DVE_COLS, NSUM, RES2_COLS = _layout()

_CACHE = {}


def _build_bass():
    import concourse.mybir as mybir
    from concourse import bacc
    from concourse.tile import TileContext

    f32 = mybir.dt.float32
    f32r = mybir.dt.float32r
    f16 = mybir.dt.float16

    nc = bacc.Bacc("TRN2", num_devices=NCORES)
    zaT = nc.dram_tensor("zaT", [D, A], f16, kind="ExternalInput")
    ztT = nc.dram_tensor("ztT", [D, MLOC], f16, kind="ExternalInput")
    vtT = nc.dram_tensor("vtT", [D, MLOC], f32, kind="ExternalInput")
    res = nc.dram_tensor("res", [128, NSUM], f32, kind="ExternalOutput")
    res2 = nc.dram_tensor("res2", [128, RES2_COLS], f32, kind="ExternalOutput")

    with TileContext(nc) as tc:
        with (
            tc.tile_pool(name="persist", bufs=1) as persist,
            tc.tile_pool(name="scratch", bufs=2) as scratch,
            tc.tile_pool(name="scratch3", bufs=4) as scratch3,
            tc.tile_pool(name="psum", bufs=2, space="PSUM") as psum_pool,
        ):
            # DMA order follows the critical chain: the jb=0 weight slice
            # (32KB) lands first, then the first matmul group's rhs, then the
            # rest. HWDGE issue cost is ~constant per DMA, so few big
            # transfers win over per-chunk loads.
            zt_tile = persist.tile([128, MLOC], f16, tag="zt")
            nc.sync.dma_start(out=zt_tile[:, 0:128], in_=ztT[:, 0:128])
            za_tiles = []
            t0 = persist.tile([128, GRP * ACH], f16, tag="za0")
            nc.sync.dma_start(out=t0[:, 0:1024], in_=zaT[:, 0:1024])
            nc.sync.dma_start(out=t0[:, 1024:2048], in_=zaT[:, 1024:2048])
            za_tiles.append(t0)
            res2_tile = persist.tile([128, RES2_COLS], f32, tag="res2")
            t1 = persist.tile([128, GRP * ACH], f16, tag="za1")
            nc.sync.dma_start(out=t1[:, 0:1024], in_=zaT[:, 2048:3072])
            nc.sync.dma_start(out=t1[:, 1024:2048], in_=zaT[:, 3072:4096])
            za_tiles.append(t1)
            for g in range(2, NGRP):
                t = persist.tile([128, GRP * ACH], f16, tag=f"za{g}")
                nc.sync.dma_start(
                    out=t[:], in_=zaT[:, g * GRP * ACH:(g + 1) * GRP * ACH]
                )
                za_tiles.append(t)
            # zt columns beyond jb=0 are first consumed ~15us in; load last
            nc.sync.dma_start(out=zt_tile[:, 128:MLOC], in_=ztT[:, 128:MLOC])
            res_tile = persist.tile([128, NSUM], f32, tag="res")

            def emit_group(jb, g):
                use_dve = (jb, g) in _DVE_COLS
                sumcol, nh = (_DVE_COLS if use_dve else _ACC_COLS)[(jb, g)]
                lhsT = zt_tile[:, jb * 128:(jb + 1) * 128]
                ps = psum_pool.tile([128, GRP * ACH], f32, tag="ps")
                for k in range(GRP):
                    nc.tensor.matmul(
                        ps[:, k * ACH:(k + 1) * ACH],
                        lhsT,
                        za_tiles[g][:, k * ACH:(k + 1) * ACH],
                        start=True,
                        stop=True,
                    )
                pool = scratch3 if use_dve else scratch
                scr = pool.tile([128, GRP * ACH], f32,
                                tag="expscrD" if use_dve else "expscr")
                # Early groups' ACT is subdivided so the exp stream starts
                # before the whole first rhs region has arrived.
                w = GRP * ACH // nh
                for h in range(nh):
                    nc.scalar.activation(
                        out=scr[:, h * w:(h + 1) * w],
                        in_=ps[:, h * w:(h + 1) * w],
                        func=mybir.ActivationFunctionType.Exp,
                        scale=1.0 / TEMPERATURE,
                        accum_out=None if use_dve
                        else res_tile[:, sumcol + h:sumcol + h + 1],
                    )
                    if use_dve:
                        nc.vector.tensor_reduce(
                            out=res2_tile[:, sumcol + h:sumcol + h + 1],
                            in_=scr[:, h * w:(h + 1) * w],
                            axis=mybir.AxisListType.X,
                            op=mybir.AluOpType.add,
                        )

            emit_group(0, 0)

            # num partial (independent; emitted early so its DMA + DVE work
            # happen in the shadow of the exp stream): sum_j (v'_j . z_tj)
            # reduced along the free axis; partition (D) axis summed on host.
            vt_tile = persist.tile([128, MLOC], f32, tag="vt")
            nc.sync.dma_start(out=vt_tile[:], in_=vtT[:, :])
            zt_f32 = scratch.tile([128, MLOC], f32, tag="ztf32")
            nc.vector.tensor_copy(out=zt_f32[:], in_=zt_tile[:])
            num_scr = scratch.tile([128, MLOC], f32, tag="numscr")
            nc.vector.tensor_mul(out=num_scr[:], in0=vt_tile[:], in1=zt_f32[:])
            nc.vector.tensor_reduce(
                out=res2_tile[:, 0:1],
                in_=num_scr[:],
                axis=mybir.AxisListType.X,
                op=mybir.AluOpType.add,
            )

            for jb in range(NJB):
                for g in range(NGRP):
                    if jb == 0 and g == 0:
                        continue
                    emit_group(jb, g)

            nc.sync.dma_start(out=res2[:, :], in_=res2_tile[:])
            if NSUM > 2:
                nc.sync.dma_start(out=res[:, :NSUM - 2], in_=res_tile[:, :NSUM - 2])
                nc.sync.dma_start(out=res[:, NSUM - 2:], in_=res_tile[:, NSUM - 2:])
            else:
                nc.sync.dma_start(out=res[:, :], in_=res_tile[:])
    nc.compile()
    return nc


def _get_nc():
    if "nc" not in _CACHE:
        _CACHE["nc"] = _build_bass()
    return _CACHE["nc"]


def kernel(z_source, z_target, seg_source, seg_target):
    from concourse.bass_utils import run_bass_kernel_spmd

    zs = np.ascontiguousarray(z_source, dtype=np.float32)
    zt = np.ascontiguousarray(z_target, dtype=np.float32)
    seg_s = np.asarray(seg_source).astype(np.int64)
    seg_t = np.asarray(seg_target).astype(np.int64)

    # Host-side sharding prep (O(N*D), trivial next to the O(A*M*D) device work)
    # z inputs are L2-normalized (|z| <= 1), so fp16 quantization (~2.4e-4
    # rel) keeps the exp-sum well within fp32 reference noise while halving
    # the DMA volume.
    za = np.concatenate([zt, zs], axis=0)            # [A, D]
    zaT = np.ascontiguousarray(za.T.astype(np.float16))   # [D, A] fp16
    counts = np.bincount(seg_s, minlength=G).astype(np.float32)
    S = np.zeros((G, D), np.float32)
    np.add.at(S, seg_s, zs)
    v = S[seg_t] / (counts[seg_t] * np.float32(TEMPERATURE))[:, None]  # [M, D]
    vT = np.ascontiguousarray(v.T)                   # [D, M]

    in_maps = []
    for c in range(NCORES):
        j0 = c * MLOC
        in_maps.append({
            "zaT": zaT,
            "ztT": np.ascontiguousarray(zaT[:, j0:j0 + MLOC]),
            "vtT": np.ascontiguousarray(vT[:, j0:j0 + MLOC]),
        })

    nc = _get_nc()
    out = run_bass_kernel_spmd(nc, in_maps, core_ids=list(range(NCORES)))
    results = out.results

    # Host finish (float64): den = sum exp - exp(self); loss = sum log(den) - num.
    # The self dot replicates the device matmul bit-closely: fp16 inputs make
    # each product exact in fp32, and np.sum's fp32 pairwise accumulation
    # lands within ~2 ulp of the PE's accumulator (verified on hardware).
    h = zaT[:, :M].astype(np.float32)                # quantized z_target, [D, M]
    self_dot = np.sum(h * h, axis=0, dtype=np.float32).astype(np.float64)
    loss = 0.0
    for c in range(NCORES):
        r = results[c]["res"].astype(np.float64)     # [128, NSUM]
        r2 = results[c]["res2"].astype(np.float64)   # [128, 1]
        colsum = np.zeros((128, NJB))
        for jb in range(NJB):
            for g in range(NGRP):
                if (jb, g) in _DVE_COLS:
                    c0, n = _DVE_COLS[(jb, g)]
                    colsum[:, jb] += r2[:, c0:c0 + n].sum(axis=1)
                else:
                    c0, n = _ACC_COLS[(jb, g)]
                    colsum[:, jb] += r[:, c0:c0 + n].sum(axis=1)
        jj = c * MLOC + np.arange(NJB)[None, :] * 128 + np.arange(128)[:, None]
        den = colsum - np.exp(self_dot[jj] / TEMPERATURE)
        loss += np.sum(np.log(den))
        loss -= r2[:, 0].sum()
    return np.asarray(loss, dtype=np.float32)



# revision 6
# speedup vs baseline: 1.2867x; 1.2867x over previous
"""Trainium2 Bass kernel for a grouped contrastive loss.

Math (matches the reference):
    z_a = concat(z_target, z_source)                      # [A=M+N, D]
    den[j]  = sum_a exp((z_a[a].z_target[j]) / T) - exp(z_tj.z_tj / T)
    num[j]  = mean_{s: seg_source[s]==seg_target[j]} (z_s . z_tj) / T
    loss = sum_j log(den[j]) - num[j]

Device computes only the exp column-sums (the O(A*M) part); the num term
and the final log/sum run on the host in float64 (O(M*D), trivial).

Sharding: target columns j split across 8 cores (512 each); z_a replicated.
Per core the [8192 x 512] sim matrix is processed as 32 units of
[128 cols x 1024 a] fed by fp8e4m3 DoubleRow matmuls (2x PE throughput,
robust to p-state ramping). PSUM can only be drained by ACT and DVE
(GPSIMD can't access PSUM, DMA can't read it), so the exp stream splits:
  - ACT:  exact Exp activation with accum_out (17 units, incl. the 4
          diagonal units).
  - DVE:  Schraudolph fp16 exp pass1 for the other 15 units:
          tensor_scalar affine -> int16 (PSUM -> SBUF).
  - The summing pass2 over the bitcast-f16 approx values runs mostly on
    Pool/GPSIMD (14 units, SBUF-only is fine there) and one unit's pass2
    on DVE's 4x perf mode. Same-jb unit pairs share one 2048-wide pass2.
Schraudolph error is ~+-4% per term, quasi-random with a mean-zero
offset constant; den averages ~1700 effective terms so den error lands
around 0.1-0.3% and the summed loss error ~1e-4 (measured 1.4e-4).

The self term exp(z_tj.z_tj/T) ~ 1.6e6 dwarfs den ~ 1.8e4, so its unit
must be exact and host-replicable: the per-core z_a row block is swapped
so each core's own 512 target rows sit at a in [0,512) -- the diagonal
then always lives in the g=0 units, which are pinned to ACT. The host
subtracts exp of the bit-replicated fp8 self-dot (fp8 products are exact
in fp32; pairwise np.sum matches the PE adder tree to ~2 ulp).
"""

import numpy as np

TEMPERATURE = 0.07
N = 4096       # z_source rows
M = 4096       # z_target rows
D = 128        # embedding dim
G = 64         # groups
NCORES = 8
MLOC = M // NCORES          # 512 target columns per core
A = M + N                   # 8192 rows of z_a
UW = 1024                   # unit width along a
NJB = MLOC // 128           # 4 column blocks of 128
NG = A // UW                # 8 a-chunks
N_ACT, N_POOL, N_DVE = 18, 0, 14   # GPSIMD cannot touch PSUM nor run TSP; 2-engine split

# Schraudolph fp16 constants: exp(r/T) ~= bitcast_f16(int16(r*S1 + B16))
_A16 = 1024.0 * np.float32(np.log2(np.e))
_C16 = 1024.0 * np.log2(1.0406)       # mean-zero offset (uniform-fraction)
B16 = float(np.float32(15.0 * 1024.0 - _C16))
S1 = float(np.float32(_A16 / TEMPERATURE))


def _schedule():
    """Engine slot sequence + unit assignment + accum-column layout.

    slots[i] in 'APD': A = ACT unit; P/D = approx unit whose pass2 goes
    to Pool/DVE. units[i] = (jb, g); diagonal units (g=0) pinned to ACT.
    cols[i]: res_a column for A slots, res_d column for P/D (both halves
    of a same-jb pair share one column; odd counts end with a single).
    """
    targets = {"A": N_ACT, "P": N_POOL, "D": N_DVE}
    acc = {"A": 0.0, "P": 0.0, "D": 0.0}
    slots = []
    for _ in range(32):
        for e in "APD":
            acc[e] += targets[e]
        pick = max("APD", key=lambda e: acc[e])
        acc[pick] -= 32.0
        slots.append(pick)

    # per-path unit queues; P/D queues are flattened same-jb pairs
    per_jb = {jb: [(jb, g) for g in range(1, NG)] for jb in range(NJB)}

    def pairs_for(n, jbs):
        q = []
        for jb in jbs[:n // 2]:
            q += [per_jb[jb].pop(0), per_jb[jb].pop(0)]
        if n % 2:
            jb = max(per_jb, key=lambda b: len(per_jb[b]))
            q.append(per_jb[jb].pop(0))
        return q

    p_queue = pairs_for(N_POOL, [])
    d_queue = pairs_for(N_DVE, [0, 0, 1, 1, 2, 2, 3])
    a_queue = [(jb, 0) for jb in range(NJB)] + sorted(
        (u for lst in per_jb.values() for u in lst), key=lambda u: u[1])
    queues = {"A": a_queue, "P": p_queue, "D": d_queue}
    remaining = {e: targets[e] for e in "APD"}

    units, cols = [], []
    ca, cd = 0, 0
    pend = {"P": False, "D": False}   # waiting for second of pair
    for e in slots:
        units.append(queues[e].pop(0))
        remaining[e] -= 1
        if e == "A":
            cols.append(ca)
            ca += 1
        else:
            cols.append(cd)           # pair shares one accum column
            last_single = (not pend[e]) and remaining[e] == 0
            if pend[e] or last_single:
                cd += 1
                pend[e] = False
            else:
                pend[e] = True
    assert ca == N_ACT and all(not q for q in queues.values())
    assert not pend["P"] and not pend["D"]
    return slots, units, cols, cd


SLOTS, UNITS, COLS, ND_COLS = _schedule()

_CACHE = {}


def _build_bass():
    import concourse.mybir as mybir
    from concourse import bacc
    from concourse.tile import TileContext

    f32 = mybir.dt.float32
    f16 = mybir.dt.float16
    f8 = mybir.dt.float8e4
    i16 = mybir.dt.int16
    Alu = mybir.AluOpType
    Act = mybir.ActivationFunctionType
    DR = mybir.MatmulPerfMode.DoubleRow

    nc = bacc.Bacc("TRN2", num_devices=NCORES)
    za8 = nc.dram_tensor("za8", [64, 2, A], f8, kind="ExternalInput")
    wt8 = nc.dram_tensor("wt8", [64, 2, MLOC], f8, kind="ExternalInput")
    res_a = nc.dram_tensor("res_a", [128, N_ACT], f32, kind="ExternalOutput")
    res_d = nc.dram_tensor("res_d", [128, ND_COLS], f32, kind="ExternalOutput")

    remaining = {"P": N_POOL, "D": N_DVE}

    with TileContext(nc) as tc:
        with (
            tc.tile_pool(name="persist", bufs=1) as persist,
            tc.tile_pool(name="ascr", bufs=2) as ascr_pool,
            tc.tile_pool(name="dscr", bufs=2) as dscr_pool,
            tc.tile_pool(name="pscr", bufs=2) as pscr_pool,
            tc.tile_pool(name="junk", bufs=2) as junk_pool,
            tc.tile_pool(name="psum", bufs=4, space="PSUM") as psum_pool,
        ):
            wt_t = persist.tile([64, 2, MLOC], f8, tag="wt")
            nc.sync.dma_start(out=wt_t[:], in_=wt8[:, :, :])
            za_t = persist.tile([64, 2, A], f8, tag="za")
            seen = set()
            for (_, g) in UNITS:
                if g not in seen:
                    seen.add(g)
                    nc.sync.dma_start(
                        out=za_t[:, :, g * UW:(g + 1) * UW],
                        in_=za8[:, :, g * UW:(g + 1) * UW],
                    )
            resa_t = persist.tile([128, N_ACT], f32, tag="ra")
            resd_t = persist.tile([128, ND_COLS], f32, tag="rd")

            half = {"P": None, "D": None}   # scr tile holding pair's 1st half
            for slot in range(32):
                e = SLOTS[slot]
                jb, g = UNITS[slot]
                col = COLS[slot]
                ps = psum_pool.tile([128, UW], f32, tag="ps")
                for k in range(2):
                    nc.tensor.matmul(
                        ps[:, k * 512:(k + 1) * 512],
                        wt_t[:, 0:2, jb * 128:(jb + 1) * 128],
                        za_t[:, 0:2, g * UW + k * 512:g * UW + (k + 1) * 512],
                        start=True,
                        stop=True,
                        perf_mode=DR,
                    )
                if e == "A":
                    scrf = ascr_pool.tile([128, UW], f32, tag="ascr")
                    nc.scalar.activation(
                        out=scrf[:],
                        in_=ps[:],
                        func=Act.Exp,
                        scale=1.0 / TEMPERATURE,
                        accum_out=resa_t[:, col:col + 1],
                    )
                    continue
                # approx: DVE affine pass1 -> i16 half of a (possibly) paired scr
                remaining[e] -= 1
                spool = pscr_pool if e == "P" else dscr_pool
                stag = "pscr" if e == "P" else "dscr"
                if half[e] is None:
                    single = remaining[e] == 0   # last unit unpaired
                    w = UW if single else 2 * UW
                    scr = spool.tile([128, w], i16, tag=stag + str(w))
                    lo = 0
                    if not single:
                        half[e] = scr
                else:
                    scr = half[e]
                    lo = UW
                    half[e] = None
                nc.vector.tensor_scalar(
                    out=scr[:, lo:lo + UW], in0=ps[:],
                    scalar1=S1, scalar2=B16,
                    op0=Alu.mult, op1=Alu.add,
                )
                if half[e] is None:   # pair (or single) complete: pass2
                    w = scr.shape[-1]
                    junk = junk_pool.tile([128, w], f16, tag="junk" + str(w))
                    nc.vector.tensor_scalar(
                        out=junk[:], in0=scr[:].bitcast(f16),
                        scalar1=1.0, scalar2=0.0,
                        op0=Alu.mult, op1=Alu.add,
                        accum_out=resd_t[:, col:col + 1],
                    )

            nc.sync.dma_start(out=res_a[:, :], in_=resa_t[:])
            nc.sync.dma_start(out=res_d[:, :], in_=resd_t[:])
    nc.compile()
    return nc


def _get_nc():
    if "nc" not in _CACHE:
        _CACHE["nc"] = _build_bass()
    return _CACHE["nc"]


def _prep_inputs(z_source, z_target):
    """fp8-quantize, build per-core DoubleRow layouts (row-swapped)."""
    import ml_dtypes

    zs = np.ascontiguousarray(z_source, dtype=np.float32)
    zt = np.ascontiguousarray(z_target, dtype=np.float32)
    za = np.concatenate([zt, zs], axis=0)                  # [A, D]
    za8 = za.astype(ml_dtypes.float8_e4m3)                 # [A, D] fp8
    # DoubleRow layout: lay[p, h, a] = za8[a, 64h + p]
    lay0 = np.ascontiguousarray(
        za8.T.reshape(2, 64, A).transpose(1, 0, 2))        # [64, 2, A]
    in_maps = []
    for c in range(NCORES):
        lay = lay0.copy()
        if c != 0:
            lay[:, :, 0:MLOC] = lay0[:, :, c * MLOC:(c + 1) * MLOC]
            lay[:, :, c * MLOC:(c + 1) * MLOC] = lay0[:, :, 0:MLOC]
        wt = np.ascontiguousarray(lay0[:, :, c * MLOC:(c + 1) * MLOC])
        in_maps.append({"za8": np.ascontiguousarray(lay), "wt8": wt})
    return za8, in_maps


def kernel(z_source, z_target, seg_source, seg_target):
    from concourse.bass_utils import run_bass_kernel_spmd

    zs = np.ascontiguousarray(z_source, dtype=np.float32)
    zt = np.ascontiguousarray(z_target, dtype=np.float32)
    seg_s = np.asarray(seg_source).astype(np.int64)
    seg_t = np.asarray(seg_target).astype(np.int64)

    za8, in_maps = _prep_inputs(zs, zt)

    nc = _get_nc()
    out = run_bass_kernel_spmd(nc, in_maps, core_ids=list(range(NCORES)))
    results = out.results

    # Host finish in float64.
    # num term, exact from the unquantized inputs:
    counts = np.bincount(seg_s, minlength=G).astype(np.float64)
    Sg = np.zeros((G, D), np.float64)
    np.add.at(Sg, seg_s, zs.astype(np.float64))
    v = Sg[seg_t] / (counts[seg_t] * TEMPERATURE)[:, None]
    num_total = float(np.sum(v * zt.astype(np.float64)))

    # self dots, replicating the PE's DoubleRow fp8 accumulation:
    q = za8[:M].astype(np.float32)                         # [M, D]
    self_r = (np.sum(q[:, :64] * q[:, :64], axis=1, dtype=np.float32)
              + np.sum(q[:, 64:] * q[:, 64:], axis=1, dtype=np.float32))
    self_term = np.exp(self_r.astype(np.float64) / TEMPERATURE)

    loss = 0.0
    for c in range(NCORES):
        ra = results[c]["res_a"].astype(np.float64)        # [128, N_ACT]
        rd = results[c]["res_d"].astype(np.float64)
        colsum = np.zeros((128, NJB))
        seen_d = set()
        for slot in range(32):
            jb, _ = UNITS[slot]
            col = COLS[slot]
            if SLOTS[slot] == "A":
                colsum[:, jb] += ra[:, col]
            else:
                key = (SLOTS[slot], col)
                if key not in seen_d:   # pair shares one accum column
                    seen_d.add(key)
                    colsum[:, jb] += rd[:, col]
        jj = c * MLOC + np.arange(NJB)[None, :] * 128 + np.arange(128)[:, None]
        den = colsum - self_term[jj]
        loss += np.sum(np.log(den))
    loss -= num_total
    return np.asarray(loss, dtype=np.float32)
